# revision 1
# baseline (speedup 1.0000x reference)
"""Distributed sparse-MoE (top-1 routing, shared expert FFN) for 8 trn2 NeuronCores.

Math: reference computes
    logits = hidden @ Wg + bg ; probs = softmax(logits)
    best   = argmax(probs)    ; order = stable argsort(best)
    out[t] = (hidden[order[t]] @ We + be) * probs[t, best[t]]

Since every expert shares the same FFN weight `We`, the dispatch permutation
commutes with the matmul:  (hidden[order]) @ We = (hidden @ We)[order].
So each core runs the dense FFN matmul on a contiguous 2048-token shard in
ORIGINAL token order (no all-to-all needed); the router gate is a second tiny
matmul over the same token slabs (stationary Wg, 8 output partitions).  The
host applies the data-dependent permutation + top-1 probability scale while
gathering the 8 shards back into the full output.

Device work per core: [2048, 2048] @ [2048, 2048] FFN + [2048, 2048] @
[2048, 8] gate, both in float32r (tf32 inputs, fp32 accumulate, full PE
rate).  tf32 gate safety was verified against the reference's fp32 argmax on
the actual (seeded) inputs: 0/16384 flips, min tf32 top-2 logit gap 5.8e-5
vs ~3e-6 accumulation-order noise.
"""

import os

import numpy as np

import concourse.bacc as bacc
import concourse.bass as bass
import concourse.mybir as mybir
import concourse.tile as tile
from concourse.bass_utils import run_bass_kernel_spmd

# Problem shape (hardcoded per contract).
B, S, H, E = 4, 4096, 2048, 8
T = B * S            # 16384 tokens
NCORES = 8
TPC = T // NCORES    # 2048 tokens per core
P = 128              # partitions
KT = H // P          # 16 contraction blocks
NW = 512             # matmul moving free-dim (one PSUM bank of fp32)
NMAIN = H // NW      # 4 main n-groups
SLAB = 256           # tokens per x DMA slab (2 m-subtiles)

# Main-matmul dtype: "f32r" (tf32, full PE rate), "f32" (4x slower, exact),
# "bf16" (full rate, ~4e-3 rel err).
DT_MAIN = os.environ.get("MOE_DT", "f32r")
# "device": gate matmul on-device (f32r).  "host": numpy fp32 gate.
GATE = os.environ.get("MOE_GATE", "device")


def _round_tf32(a: np.ndarray) -> np.ndarray:
    """Round fp32 to tf32 (10-bit mantissa), round-to-nearest-even."""
    u = np.ascontiguousarray(a, dtype=np.float32).view(np.uint32)
    r = (u + np.uint32(0xFFF) + ((u >> np.uint32(13)) & np.uint32(1))) & np.uint32(
        0xFFFFE000
    )
    return r.view(np.float32)


def _build(dt_main: str, gate_device: bool) -> bass.Bass:
    # Bacc (not raw Bass): its compile() runs generate_event_semaphores,
    # which splits multi-waits to satisfy TRN2's 1-wait-per-instruction
    # hardware constraint.
    nc = bacc.Bacc(None, target_bir_lowering=False)
    f32 = mybir.dt.float32
    f32r = mybir.dt.float32r
    bf16 = mybir.dt.bfloat16
    mm_dt = {"f32r": f32r, "f32": f32, "bf16": bf16}[dt_main]

    # xr: tokens pre-rounded on host to the matmul dtype's precision.
    xr = nc.dram_tensor("xr", [H, TPC], mm_dt, kind="ExternalInput")
    wm = nc.dram_tensor("wm", [H, H], mm_dt, kind="ExternalInput")
    bc = nc.dram_tensor("bc", [1, H], f32, kind="ExternalInput")
    if gate_device:
        wg = nc.dram_tensor("wg", [H, E], mm_dt, kind="ExternalInput")
        bg = nc.dram_tensor("bg", [E, 1], f32, kind="ExternalInput")
        yg = nc.dram_tensor("yg", [E, TPC], f32, kind="ExternalOutput")
    else:
        sc = nc.dram_tensor("sc", [TPC, 1], f32, kind="ExternalInput")
    yo = nc.dram_tensor("yo", [TPC, H], f32, kind="ExternalOutput")

    xr_r = xr[:].rearrange("(ko ki) t -> ki ko t", ki=P)   # [128, KT, TPC]
    wm_r = wm[:].rearrange("(ko ki) n -> ki ko n", ki=P)   # [128, KT, H]

    with tile.TileContext(nc) as tc:
        with (
            tc.tile_pool(name="wpool", bufs=1) as wpool,
            tc.tile_pool(name="cpool", bufs=1) as cpool,
            tc.tile_pool(name="rpool", bufs=3) as rpool,
            tc.tile_pool(name="opool", bufs=2) as opool,
            tc.tile_pool(name="ogpool", bufs=2) as ogpool,
            tc.tile_pool(name="spool", bufs=4) as spool,
            tc.tile_pool(name="pspool", bufs=7, space="PSUM") as pspool,
            tc.tile_pool(name="psgpool", bufs=1, space="PSUM") as psgpool,
        ):
            # Bias row tile (replicated to all 128 partitions by a 0-stride
            # DMA emitted in the preload sequence below).
            b_sb = cpool.tile([P, H], f32)

            n_slabs = TPC // SLAB
            subs = SLAB // P
            PHA = min(2, n_slabs)  # slabs resident during the warmup phase

            def _fetch_slab(m):
                t = rpool.tile([P, KT, SLAB], mm_dt, tag="xm")
                nc.sync.dma_start(out=t, in_=xr_r[:, :, m * SLAB : (m + 1) * SLAB])
                return t

            def _fetch_scale(m):
                t = spool.tile([P, subs], f32, tag="s")
                nc.sync.dma_start(
                    out=t,
                    in_=sc[m * SLAB : (m + 1) * SLAB, :].rearrange(
                        "(s p) o -> p (s o)", p=P
                    ),
                )
                return t

            def do_group(xm, s_m, m, sub, n):
                ps = pspool.tile([P, NW], f32, tag="ps")
                for k in range(KT):
                    nc.tensor.matmul(
                        ps,
                        xm[:, k, sub * P : (sub + 1) * P],
                        w_sb[:, k, n * NW : (n + 1) * NW],
                        start=(k == 0),
                        stop=(k == KT - 1),
                    )
                o_sb = opool.tile([P, NW], f32, tag="o")
                nc.vector.tensor_add(
                    out=o_sb, in0=ps, in1=b_sb[:, n * NW : (n + 1) * NW]
                )
                if s_m is not None:
                    nc.vector.tensor_scalar_mul(
                        out=o_sb, in0=o_sb, scalar1=s_m[:, sub : sub + 1]
                    )
                t0 = (m * subs + sub) * P
                nc.sync.dma_start(
                    out=yo[t0 : t0 + P, n * NW : (n + 1) * NW], in_=o_sb
                )

            def do_gate(xm, m):
                # Gate: stationary Wg (8 cols), moving = the whole token
                # slab.  Output is logits^T [E, SLAB].
                psg = psgpool.tile([E, SLAB], f32, tag="psg")
                for k in range(KT):
                    nc.tensor.matmul(
                        psg,
                        wg_sb[:, k, :],
                        xm[:, k, :],
                        start=(k == 0),
                        stop=(k == KT - 1),
                    )
                og = ogpool.tile([E, SLAB], f32, tag="og")
                nc.vector.tensor_scalar(
                    out=og,
                    in0=psg,
                    scalar1=bg_sb,
                    scalar2=None,
                    op0=mybir.AluOpType.add,
                )
                nc.sync.dma_start(out=yg[:, m * SLAB : (m + 1) * SLAB], in_=og)

            # DMA order: W chunk 0 and slab 0 split into k-halves (PE's first
            # 8-deep half-group can start after ~half the bytes), bias, slab
            # 1, W chunks 1..3, gate weights, prefetched slab 2.  The n-outer
            # warmup below gives PE chunk-0-only work while chunks 1..3 land,
            # so no dispatch ever blocks on W.
            # PE warmup/bridge bursts: dependency-free bf16 matmuls on a
            # memset tile keep the tensor engine busy (and the HAM pstate
            # warm) across DMA-wait windows where no real matmul is ready.
            dum = cpool.tile([P, 128], mybir.dt.bfloat16)
            nc.vector.memset(dum, 1.0)
            dps = psgpool.tile([P, 128], f32, tag="psg", name="dps")

            def warm(count):
                for _ in range(count):
                    nc.tensor.matmul(dps, dum, dum, start=True, stop=True)

            warm(36)

            # Gate weights first (tiny): phase-A gates then run during the
            # W-stream windows where no main matmul is ready.
            if gate_device:
                wg_sb = wpool.tile([P, KT, E], mm_dt)
                nc.sync.dma_start(
                    out=wg_sb, in_=wg[:].rearrange("(ko ki) e -> ki ko e", ki=P)
                )
                bg_sb = cpool.tile([E, 1], f32)
                nc.sync.dma_start(out=bg_sb, in_=bg[:])

            KH = KT // 2
            w_sb = wpool.tile([P, KT, H], mm_dt)
            xm0 = rpool.tile([P, KT, SLAB], mm_dt, tag="xm", name="xm0")
            # First W chunk + first slab interleaved in fine k-pieces (finest
            # first): PE's first accumulation group starts after ~an eighth
            # of the bytes.
            for klo, khi in ((0, 2), (2, 4), (4, 8), (8, 12), (12, 16)):
                ksl = slice(klo, khi)
                nc.sync.dma_start(out=w_sb[:, ksl, :NW], in_=wm_r[:, ksl, :NW])
                nc.sync.dma_start(out=xm0[:, ksl, :], in_=xr_r[:, ksl, :SLAB])
            xms = {0: xm0}
            for m in range(1, PHA):
                xms[m] = _fetch_slab(m)
            bias_bcast = bass.AP(tensor=bc, offset=0, ap=[[0, P], [1, H]])
            nc.sync.dma_start(out=b_sb, in_=bias_bcast)
            scs = {}
            if not gate_device:
                for m in range(PHA):
                    scs[m] = _fetch_scale(m)
            # Remaining W chunks in k-halves so each n-group can begin on
            # half-K as soon as the first half lands.
            for n in range(1, NMAIN):
                nsl = slice(n * NW, (n + 1) * NW)
                nc.sync.dma_start(out=w_sb[:, :KH, nsl], in_=wm_r[:, :KH, nsl])
                nc.sync.dma_start(out=w_sb[:, KH:, nsl], in_=wm_r[:, KH:, nsl])
            # Early prefetch of the first steady-state slab (own pool slot).
            if n_slabs > PHA:
                xm_next = _fetch_slab(PHA)
                sc_next = _fetch_scale(PHA) if not gate_device else None

            # Phase A: gates first (they only need the slab + wg, filling the
            # early W-stream idle), then the main groups n-outer over the
            # resident warmup slabs.
            if gate_device:
                for m in range(PHA):
                    do_gate(xms[m], m)
            for n in range(NMAIN):
                for m in range(PHA):
                    for sub in range(subs):
                        do_group(xms[m], scs.get(m), m, sub, n)

            # Phase B: steady-state, slab-major, software-pipelined prefetch.
            for m in range(PHA, n_slabs):
                xm, s_m = xm_next, sc_next
                if m + 1 < n_slabs:
                    xm_next = _fetch_slab(m + 1)
                    sc_next = _fetch_scale(m + 1) if not gate_device else None
                for sub in range(subs):
                    for n in range(NMAIN):
                        do_group(xm, s_m, m, sub, n)
                if gate_device:
                    do_gate(xm, m)
    nc.compile()
    return nc


_NC_CACHE: dict = {}


def _get_nc(dt_main: str, gate_device: bool) -> bass.Bass:
    key = (dt_main, gate_device)
    if key not in _NC_CACHE:
        _NC_CACHE[key] = _build(dt_main, gate_device)
    return _NC_CACHE[key]


def _softmax_top1(logits: np.ndarray):
    """best index, top-1 softmax prob (fp32, matches jax argmax semantics)."""
    logits = np.ascontiguousarray(logits, dtype=np.float32)
    mx = logits.max(axis=1, keepdims=True)
    ex = np.exp(logits - mx, dtype=np.float32)
    denom = ex.sum(axis=1)
    best = logits.argmax(axis=1)
    best_p = ex[np.arange(logits.shape[0]), best] / denom
    return best, best_p


def _prep_mm(a: np.ndarray, dt_main: str) -> np.ndarray:
    """Prepare an operand for the main matmul's dtype (host-side rounding)."""
    if dt_main == "f32r":
        return _round_tf32(a)
    if dt_main == "bf16":
        import ml_dtypes

        return np.ascontiguousarray(a).astype(ml_dtypes.bfloat16)
    return np.ascontiguousarray(a)


def kernel(x, Wg, bg, We, be):
    x = np.asarray(x, dtype=np.float32)
    Wg = np.asarray(Wg, dtype=np.float32)
    bg = np.asarray(bg, dtype=np.float32)
    We = np.asarray(We, dtype=np.float32)
    be = np.asarray(be, dtype=np.float32)

    hidden = np.ascontiguousarray(x.reshape(T, H))
    gate_device = GATE == "device"
    nc = _get_nc(DT_MAIN, gate_device)
    wm_np = _prep_mm(We, DT_MAIN)
    bc_np = be[None, :].astype(np.float32)

    if gate_device:
        wg_np = _prep_mm(Wg, DT_MAIN)
        bg_np = np.ascontiguousarray(bg[:, None]).astype(np.float32)
        in_maps = []
        for c in range(NCORES):
            xt_c = np.ascontiguousarray(hidden[c * TPC : (c + 1) * TPC].T)
            in_maps.append(
                {
                    "xr": _prep_mm(xt_c, DT_MAIN),
                    "wm": wm_np,
                    "wg": wg_np,
                    "bc": bc_np,
                    "bg": bg_np,
                }
            )
        res = run_bass_kernel_spmd(nc, in_maps, core_ids=list(range(NCORES)))
        y = np.concatenate([r["yo"] for r in res.results], axis=0)      # [T, H]
        logits = np.concatenate([r["yg"] for r in res.results], axis=1).T
        # Tie guard: the device gate runs at tf32 precision (logit error
        # ~1e-4).  For the few tokens whose top-2 gap is within that bound,
        # recompute the logits exactly (fp64) so a near-tie can never flip
        # the argmax vs the fp32 reference and corrupt the sort permutation.
        logits = np.ascontiguousarray(logits, dtype=np.float32)
        srt = np.sort(logits, axis=1)
        suspects = np.nonzero(srt[:, -1] - srt[:, -2] < 1e-3)[0]
        if suspects.size:
            exact = (
                hidden[suspects].astype(np.float64) @ Wg.astype(np.float64)
                + bg.astype(np.float64)
            ).astype(np.float32)
            logits[suspects] = exact
        best, best_p = _softmax_top1(logits)
        order = np.argsort(best, kind="stable")
        out = y[order] * best_p[:, None]
    else:
        # Host gate: shards are the tokens PERMUTED by destination slot; the
        # device applies the top-1 scale, so shard outputs are final rows.
        logits = hidden @ Wg + bg
        best, best_p = _softmax_top1(logits)
        order = np.argsort(best, kind="stable")
        xp = hidden[order]
        in_maps = []
        for c in range(NCORES):
            xt_c = np.ascontiguousarray(xp[c * TPC : (c + 1) * TPC].T)
            sc_c = np.ascontiguousarray(best_p[c * TPC : (c + 1) * TPC, None])
            in_maps.append(
                {"xr": _prep_mm(xt_c, DT_MAIN), "wm": wm_np, "bc": bc_np, "sc": sc_c}
            )
        res = run_bass_kernel_spmd(nc, in_maps, core_ids=list(range(NCORES)))
        out = np.concatenate([r["yo"] for r in res.results], axis=0)

    return out.reshape(B, S, H).astype(np.float32)

